# revision 6
# baseline (speedup 1.0000x reference)
"""Trainium2 Bass kernel for BaseLIDIA weighted overlap-add (fold) network.

Math (derived from the reference):
  out[t,ch,y,x] = 0.5 * img[t,ch,y,x] / cnt[t,y,x] + mean(noisy[t,ch])
  img[ch,y,x]   = sum_{i,j in 0..4} deno[t, (y+4-i)*536 + (x+4-j), ch*25+i*5+j]
                                    * w[t, (y+4-i)*536 + (x+4-j)]
  cnt[y,x]      = sum_{i,j in 0..4} w[t, (y+4-i)*536 + (x+4-j)]
(`inds` is unused by the reference; the pre/post scaling collapses so that the
only use of `noisy` is its raw per-channel mean.)

Sharding: 8 cores = 2 frames x 4 row-bands of 133 output rows. Each core gets
patch rows [133b, 133b+137) (4-row halo) of its frame.

Per-core on-device algorithm (columns q on SBUF partitions):
  - load deno band tile [q=128, r=137, d=75], w^T tile [q, r]
  - WDt = deno * w          (tensor_tensor, split across DVE + GpSimd)
  - S[x, r, ch, i] = sum_j WDt[x+4-j, r, 25ch+5i+j]  as 5 PSUM-accumulated
    matmuls with 0/1 shifted-identity stationary weights (bf16, full rate)
  - img[x, ch, y] = sum_i S[x, y+4-i, ch, i]    (DVE strided tensor_reduce)
  - cnt via the same shift-matmuls on w, then 5-tap DVE reduce + reciprocal
  - finals on GpSimd, PE transpose to [ (ch,y), x ], ScalarE adds channel mean
    (bias) and the 0.5 scale during the PSUM->SBUF copy, DMA out.
"""

import ml_dtypes
import numpy as np

import concourse.bass as bass
import concourse.mybir as mybir
import concourse.tile as tile
from concourse import bacc
from concourse.bass_utils import run_bass_kernel_spmd

F32 = mybir.dt.float32
BF16 = mybir.dt.bfloat16
AX = mybir.AxisListType
ALU = mybir.AluOpType
ACTF = mybir.ActivationFunctionType

PS = 5
PH = PW = 536
H = W = 532
PD = 75
NBAND = 4
BAND_Y = 133          # output rows per band
BAND_R = 137          # patch rows per band (halo of PS-1)
RD = BAND_R * PD      # free size of a deno tile per partition (10275)
NPIX_CH = H * W       # 283024, per-channel pixel count

# x-blocks: (x0, nx, nq)  with q-range [x0, x0 + nq)
XBLKS = [(0, 124, 128), (124, 124, 128), (248, 124, 128), (372, 124, 128),
         (496, 36, 40)]
# r-chunks: (r0, nr, ny)  y-range [r0, r0+ny), needs patch rows [r0, r0+nr)
RCHUNKS = [(0, 32, 28), (28, 32, 28), (56, 32, 28), (84, 32, 28),
           (112, 25, 21)]
# transpose chunks over the flat (ch*133 + y) axis
FCHUNKS = [(0, 128), (128, 128), (256, 128), (384, 15)]
# mean_col segments: (chunk_idx, part_lo, part_hi, channel)
MEANSEG = [(0, 0, 128, 0), (1, 0, 5, 0), (1, 5, 128, 1), (2, 0, 10, 1),
           (2, 10, 128, 2), (3, 0, 15, 2)]

# fraction of the weight-multiply rows done on DVE (rest on GpSimd)
DVE_ROWS = 88


def _ap(base: bass.AP, extra_off: int, dims):
    """Custom strided view of a tile: keep the partition dim of `base`
    (optionally overriding its count), replace the free dims."""
    part = [list(base.ap[0])]
    return bass.AP(base.tensor, base.offset + extra_off, part + [list(d) for d in dims])


def _ap_p(base: bass.AP, npart: int, extra_off: int, dims):
    part = [[base.ap[0][0], npart]]
    return bass.AP(base.tensor, base.offset + extra_off, part + [list(d) for d in dims])


def build_program(reps: int = 1):
    """Build (and compile) the single-core Bass program. SPMD: all 8 cores run
    it on their own band slice. Returns the Bacc object."""
    nc = bacc.Bacc("TRN2", target_bir_lowering=False, debug=False,
                   enable_asserts=False, num_devices=8)

    deno_d = nc.dram_tensor("deno", [BAND_R, PW, PD], BF16, kind="ExternalInput")
    wt_d = nc.dram_tensor("wt", [PW, BAND_R], BF16, kind="ExternalInput")
    noisy_d = nc.dram_tensor("noisy", [3, H, W], BF16, kind="ExternalInput")
    out_d = nc.dram_tensor("out", [3, BAND_Y, W], F32, kind="ExternalOutput")

    with tile.TileContext(nc) as tc:
        with (
            tc.tile_pool(name="const", bufs=1) as const_p,
            tc.tile_pool(name="deno", bufs=2) as deno_p,
            tc.tile_pool(name="wq", bufs=2) as wq_p,
            tc.tile_pool(name="small", bufs=2) as small_p,
            tc.tile_pool(name="outp", bufs=2) as outp_p,
            tc.tile_pool(name="stage", bufs=3) as stage_p,
            tc.tile_pool(name="noisy", bufs=1) as noisy_p,
            tc.tile_pool(name="psS", bufs=5, space=bass.MemorySpace.PSUM) as psS,
            tc.tile_pool(name="psW", bufs=1, space=bass.MemorySpace.PSUM) as psW,
            tc.tile_pool(name="psT", bufs=2, space=bass.MemorySpace.PSUM) as psT,
        ):
            # ---- constants ----
            # shift identities: shifts[j][q, m] = 1 iff q == m + 4 - j
            shifts = []
            for j in range(PS):
                sh = const_p.tile([128, 124], BF16, tag=f"shift{j}")
                nc.gpsimd.memset(sh[:], 0.0)
                nc.gpsimd.affine_select(
                    out=sh[:], in_=sh[:], compare_op=ALU.not_equal, fill=1.0,
                    base=j - 4, pattern=[[-1, 124]], channel_multiplier=1)
                shifts.append(sh)
            ident = const_p.tile([124, 124], F32, tag="ident")
            nc.gpsimd.memset(ident[:], 0.0)
            nc.gpsimd.affine_select(
                out=ident[:], in_=ident[:], compare_op=ALU.not_equal, fill=1.0,
                base=0, pattern=[[-1, 124]], channel_multiplier=1)

            ones76 = const_p.tile([76, 1], BF16, tag="ones76")
            nc.gpsimd.memset(ones76[:], 1.0)
            onesrow = const_p.tile([1, 128], F32, tag="onesrow")
            nc.gpsimd.memset(onesrow[:], 1.0 / NPIX_CH)

            # ---- per-channel means of raw noisy ----
            sums = const_p.tile([1, 3], F32, tag="sums")
            for ch in range(3):
                npix = noisy_p.tile([76, 3724], BF16, tag="noisy")
                nc.sync.dma_start(
                    out=npix[:],
                    in_=bass.AP(noisy_d, ch * NPIX_CH, [[3724, 76], [1, 3724]]))
                msum = psW.tile([1, 512], F32, tag="psw")
                nchunk = (3724 + 511) // 512
                for ci in range(nchunk):
                    c0 = ci * 512
                    n = min(512, 3724 - c0)
                    nc.tensor.matmul(
                        out=msum[0:1, 0:n],
                        lhsT=ones76[:],
                        rhs=npix[:, c0:c0 + n],
                        start=(ci == 0), stop=(ci == nchunk - 1))
                nc.vector.tensor_reduce(
                    out=sums[0:1, ch:ch + 1], in_=msum[0:1, 0:512],
                    axis=AX.X, op=ALU.add)
            mrep_ps = psW.tile([128, 3], F32, tag="psw")
            nc.tensor.matmul(out=mrep_ps[:], lhsT=onesrow[:],
                             rhs=sums[:], start=True, stop=True)
            mean_rep = const_p.tile([128, 3], F32, tag="mean_rep")
            nc.scalar.copy(mean_rep[:], mrep_ps[:])
            # mean_col[p, c] = mean of channel ((128c + p) // 133)
            mean_col = const_p.tile([128, 4], F32, tag="mean_col")
            for (c, lo, hi, ch) in MEANSEG:
                # DMA: engine ops can't start at arbitrary partitions
                nc.sync.dma_start(out=mean_col[lo:hi, c:c + 1],
                                  in_=mean_rep[lo:hi, ch:ch + 1])

            # ---- main loop over x-blocks ----
            for _ in range(reps):
                for (x0, nx, nq) in XBLKS:
                    dt = deno_p.tile([128, BAND_R, PD], BF16, tag="deno")
                    # load [q, r, d]; split along r for DMA queue parallelism
                    for (r0, nr) in ((0, 28), (28, 28), (56, 28), (84, 28),
                                     (112, 25)):
                        nc.sync.dma_start(
                            out=dt[0:nq, r0:r0 + nr, :],
                            in_=bass.AP(deno_d, x0 * PD + r0 * (PW * PD),
                                        [[PD, nq], [PW * PD, nr], [1, PD]]))
                    wq = wq_p.tile([128, BAND_R], BF16, tag="wq")
                    nc.sync.dma_start(
                        out=wq[0:nq, :],
                        in_=bass.AP(wt_d, x0 * BAND_R, [[BAND_R, nq], [1, BAND_R]]))

                    # WDt = deno * w  (broadcast w over d), split DVE / GpSimd
                    dflat = dt[:]  # [128, 137, 75]
                    for eng, a, b in ((nc.vector, 0, DVE_ROWS),
                                      (nc.gpsimd, DVE_ROWS, BAND_R)):
                        eng.tensor_tensor(
                            out=_ap_p(dflat, nq, a * PD, [[PD, b - a], [1, PD]]),
                            in0=_ap_p(dflat, nq, a * PD, [[PD, b - a], [1, PD]]),
                            in1=_ap_p(wq[:], nq, a, [[1, b - a], [0, PD]]),
                            op=ALU.mult)

                    # S chunks: 5 accumulated shift-matmuls each
                    s_tiles = []
                    for (r0, nr, ny) in RCHUNKS:
                        S = psS.tile([124, 480], F32, tag="S")
                        for j in range(PS):
                            nc.tensor.matmul(
                                out=S[0:nx, 0:nr * 15],
                                lhsT=shifts[j][0:nq, 0:nx],
                                rhs=_ap_p(dflat, nq, r0 * PD + j,
                                          [[PD, nr], [PS, 15]]),
                                start=(j == 0), stop=(j == PS - 1))
                        s_tiles.append((S, r0, nr, ny))
                    Sw = psW.tile([124, BAND_R], F32, tag="psw")
                    for j in range(PS):
                        nc.tensor.matmul(
                            out=Sw[0:nx, :],
                            lhsT=shifts[j][0:nq, 0:nx],
                            rhs=wq[0:nq, :],
                            start=(j == 0), stop=(j == PS - 1))

                    # img[x, ch*133+y] via strided 5-tap reduce over i
                    outp = outp_p.tile([124, 3 * BAND_Y], F32, tag="outp")
                    for (S, r0, nr, ny) in s_tiles:
                        nc.vector.tensor_reduce(
                            out=_ap_p(outp[:], nx, r0, [[1, ny], [BAND_Y, 3]]),
                            in_=_ap_p(S[:], nx, 4, [[15, ny], [5, 3], [14, 5]]),
                            axis=AX.X, op=ALU.add)
                    # cnt + reciprocal
                    cnt = small_p.tile([124, BAND_Y], F32, tag="cnt")
                    nc.vector.tensor_reduce(
                        out=cnt[0:nx, :],
                        in_=_ap_p(Sw[:], nx, 0, [[1, BAND_Y], [1, PS]]),
                        axis=AX.X, op=ALU.add)
                    rcnt = small_p.tile([124, BAND_Y], F32, tag="rcnt")
                    nc.vector.reciprocal(rcnt[0:nx, :], cnt[0:nx, :])

                    # img *= 1/cnt (broadcast over ch)
                    nc.gpsimd.tensor_tensor(
                        out=_ap_p(outp[:], nx, 0, [[BAND_Y, 3], [1, BAND_Y]]),
                        in0=_ap_p(outp[:], nx, 0, [[BAND_Y, 3], [1, BAND_Y]]),
                        in1=_ap_p(rcnt[:], nx, 0, [[0, 3], [1, BAND_Y]]),
                        op=ALU.mult)

                    # transpose to [(ch,y), x], add mean + 0.5 scale, store
                    for c, (f0, rows) in enumerate(FCHUNKS):
                        tp = psT.tile([128, 124], F32, tag="tp")
                        nc.tensor.transpose(
                            out=tp[0:rows, 0:nx],
                            in_=outp[0:nx, f0:f0 + rows],
                            identity=ident[0:nx, 0:nx])
                        st = stage_p.tile([128, 124], F32, tag="st")
                        nc.scalar.activation(
                            st[0:rows, 0:nx], tp[0:rows, 0:nx], ACTF.Identity,
                            bias=mean_col[0:rows, c:c + 1], scale=0.5)
                        nc.sync.dma_start(
                            out=bass.AP(out_d, f0 * W + x0, [[W, rows], [1, nx]]),
                            in_=st[0:rows, 0:nx])

    nc.compile()
    return nc


_CACHE = {}


def _get_program(reps: int = 1):
    if reps not in _CACHE:
        _CACHE[reps] = build_program(reps)
    return _CACHE[reps]


def make_in_maps(noisy, deno, patch_weights):
    in_maps = []
    for core in range(8):
        t, b = divmod(core, NBAND)
        dband = deno[t].reshape(PH, PW, PD)[133 * b:133 * b + BAND_R]
        wband = patch_weights[t, :, 0].reshape(PH, PW)[133 * b:133 * b + BAND_R]
        in_maps.append({
            "deno": np.ascontiguousarray(dband).astype(ml_dtypes.bfloat16),
            "wt": np.ascontiguousarray(wband.T).astype(ml_dtypes.bfloat16),
            "noisy": np.ascontiguousarray(noisy[t]).astype(ml_dtypes.bfloat16),
        })
    return in_maps


def assemble(results):
    out = np.empty((2, 3, H, W), dtype=np.float32)
    for core in range(8):
        t, b = divmod(core, NBAND)
        out[t, :, 133 * b:133 * b + BAND_Y, :] = results[core]["out"]
    return out


def kernel(noisy, deno, patch_weights, inds=None, pixels_h=None, pixels_w=None,
           patches_h=None, patches_w=None, **_):
    noisy = np.asarray(noisy, dtype=np.float32)
    deno = np.asarray(deno, dtype=np.float32)
    patch_weights = np.asarray(patch_weights, dtype=np.float32)
    nc = _get_program()
    res = run_bass_kernel_spmd(nc, make_in_maps(noisy, deno, patch_weights),
                               core_ids=list(range(8)))
    return assemble(res.results)
